# revision 5
# baseline (speedup 1.0000x reference)
"""Causal self-attention with RoPE on 8 TRN2 NeuronCores.

Head-parallel tensor parallelism: core i owns heads 2i, 2i+1. Each core
computes its slice of the qkv projection, per-head causal attention
entirely in SBUF, and a partial output projection over its 128 channels;
a ReduceScatter sums partials and leaves each core with its 512-row
shard of the output.

All matmuls run in float32r (full PE rate, ~tf32 mantissa). Erratum
rules respected: no f32r transpose / explicit tile_position, no mixed
base partitions inside one PSUM accumulation group.
"""

import numpy as np

import concourse.bass as bass
import concourse.mybir as mybir
import concourse.tile as tile
from concourse import bacc
from concourse.bass_utils import run_bass_kernel_spmd

F32 = mybir.dt.float32
F32R = mybir.dt.float32r

B, T, C = 2, 2048, 1024
H, HD = 16, 64
NC = 8
HL = H // NC          # heads per core = 2
BT = B * T            # 4096
FQKV = 3 * HL * HD    # 384 rows of w_attn per core
TSH = BT // NC        # 512 output rows per core
NQC = T // 512        # 4 query chunks per batch
NKB = T // 128        # 16 key blocks per batch
ROPE_BASE = 10000.0

# 'gpsimd' (partition_broadcast) or 'matmul' (K=1 f32 ones matmul)
BCAST_MODE = "matmul"


def build():
    nc = bacc.Bacc(None, target_bir_lowering=False)

    xT_d = nc.dram_tensor("xT", [C, BT], F32R, kind="ExternalInput")
    wq_d = nc.dram_tensor("wqkvT", [C, FQKV], F32R, kind="ExternalInput")
    wp0_d = nc.dram_tensor("wpT0", [HD, C], F32R, kind="ExternalInput")
    wp1_d = nc.dram_tensor("wpT1", [HD, C], F32R, kind="ExternalInput")
    cos_d = nc.dram_tensor("cosT", [128, BT], F32R, kind="ExternalInput")
    sin_d = nc.dram_tensor("sinT", [128, BT], F32R, kind="ExternalInput")
    perm_d = nc.dram_tensor("permT", [128, 128], F32R, kind="ExternalInput")
    mask_d = nc.dram_tensor("masks", [4, 128, 512], F32R, kind="ExternalInput")
    id_d = nc.dram_tensor("ident", [128, 128], F32, kind="ExternalInput")
    out_d = nc.dram_tensor("out", [TSH, C], F32, kind="ExternalOutput")

    partial = nc.dram_tensor("partial", [BT, C], F32)
    rs_out = nc.dram_tensor("rs_out", [TSH, C], F32)

    with tile.TileContext(nc) as tc:
        with (
            tc.tile_pool(name="persist", bufs=1) as pp,
            tc.tile_pool(name="work", bufs=2) as wk,
            tc.tile_pool(name="pts", bufs=12) as ptp,
            tc.tile_pool(name="psA", bufs=2, space="PSUM") as psA,
            tc.tile_pool(name="psB", bufs=1, space="PSUM") as psB,
        ):
            # ---- constants / weights (persist) ----
            wq_sb = []
            for c in range(8):
                t = pp.tile([128, FQKV], F32R, name=f"wq{c}", tag=f"wq{c}")
                nc.sync.dma_start(t[:], wq_d[c * 128:(c + 1) * 128, :])
                wq_sb.append(t)
            wp_sb = []
            for hidx, w_d in enumerate((wp0_d, wp1_d)):
                t = pp.tile([HD, C], F32R, name=f"wp{hidx}", tag=f"wp{hidx}")
                nc.sync.dma_start(t[:], w_d[:])
                wp_sb.append(t)
            perm_sb = pp.tile([128, 128], F32R, name="perm_sb", tag="perm_sb")
            nc.sync.dma_start(perm_sb[:], perm_d[:])
            id_sb = pp.tile([128, 128], F32, name="id_sb", tag="id_sb")
            nc.sync.dma_start(id_sb[:], id_d[:])
            mask_sb = []
            for m in range(4):
                t = pp.tile([128, 512], F32R, name=f"mask{m}", tag=f"mask{m}")
                nc.sync.dma_start(t[:], mask_d[m])
                mask_sb.append(t)
            ones_f = pp.tile([1, HD], F32, name="ones_f", tag="ones_f")
            nc.vector.memset(ones_f[:], 1.0)
            ones_c = pp.tile([128, 1], F32, name="ones_c", tag="ones_c")
            nc.vector.memset(ones_c[:], 1.0)

            # ---- phase 1: qkvT = wqkvT.T @ xT, [f, t] layout ----
            qt = pp.tile([128, BT], F32R, name="qt", tag="qt")
            kt = pp.tile([128, BT], F32R, name="kt", tag="kt")
            vt = pp.tile([128, BT], F32, name="vt", tag="vt")
            fdst = [qt, kt, vt]
            for th in range(4):          # t quarters to bound xT residency
                xt_sb = []
                for c in range(8):
                    t = pp.tile([128, 1024], F32R, name=f"xt{th}{c}",
                                tag=f"xt{c}")
                    nc.sync.dma_start(t[:], xT_d[c * 128:(c + 1) * 128,
                                                 th * 1024:(th + 1) * 1024])
                    xt_sb.append(t)
                for f in range(3):
                    for tq in range(2):  # 512-wide psum regions
                        pq = psA.tile([128, 512], F32, name=f"pq{th}{f}{tq}",
                                      tag="ps_qkv")
                        for c in range(8):
                            nc.tensor.matmul(
                                pq[:],
                                wq_sb[c][:, f * 128:(f + 1) * 128],
                                xt_sb[c][:, tq * 512:(tq + 1) * 512],
                                start=(c == 0), stop=(c == 7),
                            )
                        off = th * 1024 + tq * 512
                        nc.scalar.copy(fdst[f][:, off:off + 512], pq[:])

            # ---- phase 2: RoPE on qt, kt (in place) ----
            for ch in range(8):
                sl = slice(ch * 512, (ch + 1) * 512)
                cosc = wk.tile([128, 512], F32R, name=f"cosc{ch}", tag="cosc")
                nc.sync.dma_start(cosc[:], cos_d[:, sl])
                sinc = wk.tile([128, 512], F32R, name=f"sinc{ch}", tag="sinc")
                nc.sync.dma_start(sinc[:], sin_d[:, sl])
                for src, dst in ((qt, qt), (kt, kt)):
                    pr = psB.tile([128, 512], F32, name=f"pr{src.name}{ch}",
                                  tag="ps_misc")
                    nc.tensor.matmul(pr[:], perm_sb[:], src[:, sl],
                                     start=True, stop=True)
                    rot = wk.tile([128, 512], F32R, name=f"rot{src.name}{ch}",
                                  tag="rot")
                    nc.scalar.copy(rot[:], pr[:])
                    tmp = wk.tile([128, 512], F32R, name=f"tmp{src.name}{ch}",
                                  tag="tmp")
                    nc.vector.tensor_mul(tmp[:], src[:, sl], cosc[:])
                    nc.vector.tensor_mul(rot[:], rot[:], sinc[:])
                    nc.vector.tensor_add(dst[:, sl], tmp[:], rot[:])

            # ---- phase 3: V blocks [t, d] with ones columns ----
            v_sb = []
            for kb in range(2 * NKB):    # 32 key blocks across both batches
                pv = psB.tile([128, 128], F32, name=f"pv{kb}", tag="ps_misc")
                nc.tensor.transpose(pv[:], vt[:, kb * 128:(kb + 1) * 128],
                                    id_sb[:])
                v = pp.tile([128, 2 * (HD + 1)], F32R, name=f"v{kb}",
                            tag=f"v{kb}")
                nc.vector.tensor_copy(v[:, 0:HD], pv[:, 0:HD])
                nc.vector.tensor_copy(v[:, HD + 1:2 * HD + 1],
                                      pv[:, HD:2 * HD])
                nc.vector.tensor_copy(v[:, HD:HD + 1], ones_c[:])
                nc.vector.tensor_copy(v[:, 2 * HD + 1:2 * HD + 2], ones_c[:])
                v_sb.append(v)

            # ---- phase 4: attention per (batch, head) ----
            # at tiles reuse xt slots (dead after phase 1)
            at_sb = [
                pp.tile([HD, BT], F32R, name=f"at{h}", tag=("xt0", "xt1")[h])
                for h in range(HL)
            ]
            for b in range(B):
                for h in range(HL):
                    hp = h * 64
                    for qc in range(NQC):
                        qsl = slice(b * T + qc * 512, b * T + (qc + 1) * 512)
                        kmax = 4 * qc + 3
                        pts = []
                        for kb in range(kmax + 1):
                            sps = psA.tile([128, 512], F32,
                                           name=f"s{b}{h}{qc}{kb}", tag="ps_s")
                            nc.tensor.matmul(
                                sps[:],
                                kt[hp:hp + 64,
                                    b * T + kb * 128:b * T + (kb + 1) * 128],
                                qt[hp:hp + 64, qsl],
                                start=True, stop=True,
                            )
                            pt = ptp.tile([128, 512], F32R,
                                          name=f"pt{b}{h}{qc}{kb}", tag="pt")
                            nc.scalar.activation(
                                pt[:], sps[:],
                                mybir.ActivationFunctionType.Exp,
                                scale=0.125,
                            )
                            if kb >= 4 * qc:
                                nc.vector.tensor_mul(
                                    pt[:], pt[:], mask_sb[kb - 4 * qc][:])
                            pts.append(pt)
                        avp = psA.tile([HD + 1, 512], F32,
                                       name=f"av{b}{h}{qc}", tag="ps_av")
                        for kb in range(kmax + 1):
                            nc.tensor.matmul(
                                avp[:],
                                v_sb[b * NKB + kb][:, h * (HD + 1):
                                                   (h + 1) * (HD + 1)],
                                pts[kb][:],
                                start=(kb == 0), stop=(kb == kmax),
                            )
                        # row HD of avp is the softmax denominator
                        rcp = wk.tile([HD + 1, 512], F32,
                                      name=f"rcp{b}{h}{qc}", tag="rcp")
                        nc.vector.reciprocal(rcp[HD:HD + 1, :],
                                             avp[HD:HD + 1, :])
                        rcp0 = wk.tile([1, 512], F32, name=f"rcp0{b}{h}{qc}",
                                       tag="rcp0")
                        nc.sync.dma_start(rcp0[:], rcp[HD:HD + 1, :])
                        bc = wk.tile([HD, 512], F32, name=f"bc{b}{h}{qc}",
                                     tag="bc")
                        if BCAST_MODE == "gpsimd":
                            nc.gpsimd.partition_broadcast(bc[:], rcp0[:])
                        else:
                            pbc = psB.tile([HD, 512], F32,
                                           name=f"pbc{b}{h}{qc}", tag="ps_bc")
                            nc.tensor.matmul(pbc[:], ones_f[:], rcp0[:],
                                             start=True, stop=True)
                            nc.scalar.copy(bc[:], pbc[:])
                        nc.vector.tensor_mul(at_sb[h][:, qsl],
                                             avp[0:HD, :], bc[:])

            # ---- phase 5: partial output projection ----
            for tb in range(BT // 128):
                for oc in range(2):
                    po = psA.tile([128, 512], F32, name=f"po{tb}{oc}",
                                  tag="ps_qkv")
                    osl = slice(oc * 512, (oc + 1) * 512)
                    nc.tensor.matmul(po[:],
                                     at_sb[0][:, tb * 128:(tb + 1) * 128],
                                     wp_sb[0][:, osl],
                                     start=True, stop=False)
                    nc.tensor.matmul(po[:],
                                     at_sb[1][:, tb * 128:(tb + 1) * 128],
                                     wp_sb[1][:, osl],
                                     start=False, stop=True)
                    st = wk.tile([128, 512], F32, name=f"st{tb}{oc}", tag="st")
                    nc.scalar.copy(st[:], po[:])
                    nc.sync.dma_start(
                        partial[tb * 128:(tb + 1) * 128, osl], st[:])

            # ---- phase 6: reduce-scatter + output ----
            nc.gpsimd.collective_compute(
                "ReduceScatter",
                mybir.AluOpType.add,
                replica_groups=[list(range(NC))],
                ins=[partial[:]],
                outs=[rs_out[:]],
            )
            nc.sync.dma_start(out_d[:], rs_out[:])

    nc.finalize()
    return nc


def host_inputs(x, w_attn, w_proj):
    """Host-side sharding/layout prep. Returns per-core in_maps."""
    x2 = np.ascontiguousarray(x.reshape(BT, C).T).astype(np.float32)  # [C,BT]

    inv = 1.0 / (ROPE_BASE ** (np.arange(0, HD, 2, dtype=np.float32) / HD))
    tpos = np.arange(T, dtype=np.float32)
    freqs = tpos[:, None] * inv[None, :]                  # [T, 32]
    emb = np.concatenate([freqs, freqs], axis=-1)         # [T, 64]
    cosT = np.cos(emb).T.astype(np.float32)               # [64, T]
    sinT = np.sin(emb).T.astype(np.float32)
    cos_full = np.ascontiguousarray(np.tile(cosT, (2, B)))  # [128, BT]
    sin_full = np.ascontiguousarray(np.tile(sinT, (2, B)))

    m64 = np.zeros((HD, HD), dtype=np.float32)
    half = HD // 2
    for d in range(half):
        m64[d, d + half] = -1.0
        m64[d + half, d] = 1.0
    perm = np.zeros((128, 128), dtype=np.float32)
    perm[0:HD, 0:HD] = m64
    perm[HD:128, HD:128] = m64
    permT = np.ascontiguousarray(perm.T)

    masks = np.zeros((4, 128, 512), dtype=np.float32)
    qi = np.arange(512)[None, :]
    ki = np.arange(128)[:, None]
    for m in range(4):
        masks[m] = (qi - ki >= m * 128).astype(np.float32)

    ident = np.eye(128, dtype=np.float32)

    in_maps = []
    for i in range(NC):
        r0 = i * (HL * HD)
        wq = w_attn[r0:r0 + HL * HD, :]
        wk_ = w_attn[C + r0:C + r0 + HL * HD, :]
        wv = w_attn[2 * C + r0:2 * C + r0 + HL * HD, :]
        wqkvT = np.ascontiguousarray(
            np.concatenate([wq, wk_, wv], axis=0).T).astype(np.float32)
        c0 = i * (HL * HD)
        wpT0 = np.ascontiguousarray(w_proj[:, c0:c0 + HD].T).astype(np.float32)
        wpT1 = np.ascontiguousarray(
            w_proj[:, c0 + HD:c0 + 2 * HD].T).astype(np.float32)
        in_maps.append({
            "xT": x2, "wqkvT": wqkvT, "wpT0": wpT0, "wpT1": wpT1,
            "cosT": cos_full, "sinT": sin_full, "permT": permT,
            "masks": masks, "ident": ident,
        })
    return in_maps


_NC_CACHE = None


def _get_nc():
    global _NC_CACHE
    if _NC_CACHE is None:
        _NC_CACHE = build()
    return _NC_CACHE


def run(x, w_attn, w_proj, trace=False):
    nc = _get_nc()
    in_maps = host_inputs(np.asarray(x), np.asarray(w_attn),
                          np.asarray(w_proj))
    res = run_bass_kernel_spmd(nc, in_maps, list(range(NC)), trace=trace)
    shards = [res.results[i]["out"] for i in range(NC)]
    out = np.concatenate(shards, axis=0).reshape(B, T, C)
    return out.astype(np.float32), res


def kernel(x, w_attn, w_proj):
    out, _ = run(x, w_attn, w_proj, trace=False)
    return out
